# revision 13
# baseline (speedup 1.0000x reference)
"""Multi-head causal attention (B=4, N=2048, D=1024, H=16) on 8 NeuronCores.

Sharding: data-parallel over batch (4) x tensor-parallel over heads (2 halves
of 8 heads each), Megatron-style.  Core c handles batch c//2 and head-half
c%2: it computes Q/K/V projections restricted to its 512 output dims, full
causal attention for its 8 heads, and a partial output projection
out_partial = ao @ Wo[:, cols].T.  The host sums the two partials per batch
(the "all-reduce") when gathering.

Device kernel (per core):
  - x transposed on PE (128x128 transpose-mode matmuls) into xT [c, n]
  - Q^T, K^T [dh, n] and V [n, dh] via float32r matmuls
  - S^T[k,q] = K Q^T per head, k-tile (128) x q-chunk (512), causal tile skip;
    2 heads issued back-to-back at partition bases 0/64 (PE row packing)
  - causal mask on diagonal tiles added in-PSUM via identity-matmul
  - softmax: exp on ScalarE from PSUM (scale=1/8 folded in); row sums via a
    ones-column appended to V (AV matmul M=65); normalization folded into the
    PSUM->SBUF copy of the attention output
  - out-projection interleaved with attention per q-chunk

`reps` replays the whole computation N times (for slope timing).
`phase` builds probe variants: "all", "p1", "attn", "exp0", "noav".
"""

import numpy as np

import concourse.bass as bass
import concourse.bacc as bacc
import concourse.mybir as mybir
import concourse.tile as tile
from concourse.bass_utils import run_bass_kernel_spmd

B, N, D, H = 4, 2048, 1024, 16
HD = 64          # head dim
DH = 512         # per-core slice of D (8 heads)
NT = N // 128    # 16 seq tiles
CT = D // 128    # 8 feature tiles
NEG = -1e30

f32 = mybir.dt.float32
f32r = mybir.dt.float32r
EXP = mybir.ActivationFunctionType.Exp


def _junk3(src_dram, rows=1024):
    """View [rows,1024] f32 dram as [128, rows//128, 1024] f32r junk."""
    return (src_dram[0:rows, :].bitcast(f32r)
            .rearrange("(o p) f -> p o f", p=128))


def _phase1(nc, tc, rep, h, phase):
    """Transposes + Q/K/V projections into h["QT"]/h["KT"]/h["Vp"]."""
    QT, KT, Vp = h["QT"], h["KT"], h["Vp"]
    with (
        tc.tile_pool(name=f"wpool{rep}", bufs=2) as wpool,
        tc.tile_pool(name=f"xtile{rep}", bufs=3) as xtile,
        tc.tile_pool(name=f"xT{rep}", bufs=2) as xT_pool,
        tc.tile_pool(name=f"ps_t{rep}", bufs=4, space="PSUM") as ps_t,
        tc.tile_pool(name=f"ps_proj{rep}", bufs=4, space="PSUM") as ps_proj,
    ):
        # ones columns of Vp (projection writes cols 0:64 of each 65-block)
        ones_f32 = wpool.tile([128, NT, 8, 1], f32, tag="ones")
        nc.vector.memset(ones_f32[:], 1.0)
        nc.vector.tensor_copy(Vp[:, :, :, 64:65], ones_f32[:])

        for name, x_in, w_in in (
            ("k", h["xk"], h["wkT"]),
            ("v", h["xv"], h["wvT"]),
            ("q", h["xq"], h["wqT"]),
        ):
            w_t = wpool.tile([128, CT, DH], f32r, tag="w")
            nc.sync.dma_start(
                w_t[:],
                w_in[:].bitcast(f32r).rearrange("(o p) f -> p o f", p=128),
            )
            for sc in range(4):  # seq chunks of 512
                xTc = xT_pool.tile([128, CT, 512], f32r, tag="xT")
                for st in range(4):  # 128-row tiles within the chunk
                    x_t = xtile.tile([128, D], f32, tag="xt")
                    row0 = sc * 512 + st * 128
                    nc.sync.dma_start(x_t[:], x_in[row0:row0 + 128, :])
                    for ct in range(CT):
                        pst = ps_t.tile([128, 128], f32, tag="pst")
                        nc.tensor.transpose(
                            pst[:], x_t[:, ct * 128:(ct + 1) * 128],
                            h["ident_t"][:],
                        )
                        dst = xTc[:, ct, st * 128:(st + 1) * 128]
                        if (st + ct) % 2:
                            nc.vector.tensor_copy(dst, pst[:])
                        else:
                            nc.scalar.copy(dst, pst[:])
                if name in ("k", "q"):
                    dstT = KT if name == "k" else QT
                    for dt_ in range(4):
                        ps = ps_proj.tile([128, 512], f32, tag="pp")
                        for ct in range(CT):
                            nc.tensor.matmul(
                                ps[:],
                                lhsT=w_t[:, ct, dt_ * 128:(dt_ + 1) * 128],
                                rhs=xTc[:, ct, :],
                                start=(ct == 0), stop=(ct == CT - 1),
                            )
                        dst = dstT[:, dt_, sc * 512:(sc + 1) * 512]
                        if dt_ % 2:
                            nc.vector.tensor_copy(dst, ps[:])
                        else:
                            nc.scalar.copy(dst, ps[:])
                else:  # v: natural layout [n, dh], strided into Vp 65-blocks
                    for st in range(4):
                        ps = ps_proj.tile([128, 512], f32, tag="pp")
                        for ct in range(CT):
                            nc.tensor.matmul(
                                ps[:],
                                lhsT=xTc[:, ct, st * 128:(st + 1) * 128],
                                rhs=w_t[:, ct, :],
                                start=(ct == 0), stop=(ct == CT - 1),
                            )
                        kt_idx = sc * 4 + st
                        src = ps[:].rearrange("p (h d) -> p h d", h=8)
                        dst = Vp[:, kt_idx, :, 0:64]
                        if st % 2:
                            nc.vector.tensor_copy(dst, src)
                        else:
                            nc.scalar.copy(dst, src)


def _phase2(nc, tc, rep, h, phase):
    """Attention + out-projection, per q-chunk."""
    QT, KT, Vp = h["QT"], h["KT"], h["Vp"]
    out, woT_t = h["out"], h["woT_t"]
    with (
        tc.tile_pool(name=f"ao{rep}", bufs=1) as ao_pool,
        tc.tile_pool(name=f"ps_s{rep}", bufs=2, space="PSUM") as ps_s,
        tc.tile_pool(name=f"ps_av{rep}", bufs=2, space="PSUM") as ps_av,
        tc.tile_pool(name=f"ps_o{rep}", bufs=2, space="PSUM") as ps_op,
        tc.tile_pool(name=f"pP{rep}", bufs=4) as pP,
        tc.tile_pool(name=f"pout{rep}", bufs=3) as pout,
        tc.tile_pool(name=f"small{rep}", bufs=4) as small,
    ):
        aoT = ao_pool.tile([128, 4, N], f32r, name="aoT")
        if phase in ("exp0", "noav"):
            # junk-fill buffers that skipped stages would have written
            nc.sync.dma_start(
                aoT[:].rearrange("p a b -> p (a b)")
                .rearrange("p (o f) -> p o f", o=8),
                _junk3(h["xq"]),
            )
            for pw in range(3):
                pwt = pP.tile([128, 2, 512], f32r, tag="p", name=f"pw{pw}")
                nc.sync.dma_start(
                    pwt[:].rearrange("p a b -> p (a b)"),
                    h["xk"][pw * 128:pw * 128 + 128, :].bitcast(f32r),
                )

        for qc in range(4):
            ktmax = qc * 4 + 4
            q0, q1 = qc * 512, (qc + 1) * 512
            for pr in range(4):  # head pairs
                if phase != "noav":
                    av = [
                        ps_av.tile([65, 512], f32, tag="av", name="av0"),
                        ps_av.tile([65, 512], f32, tag="av", name="av1"),
                    ]
                for kb in range(0, ktmax, 2):
                    nkt = min(2, ktmax - kb)
                    s_ps = [
                        ps_s.tile([128, 2, 512], f32, tag="s", name="s0"),
                        ps_s.tile([128, 2, 512], f32, tag="s", name="s1"),
                    ]
                    for kti in range(nkt):
                        kt = kb + kti
                        diag = kt >= qc * 4
                        for h2 in (0, 1):
                            p0, p1 = h2 * 64, h2 * 64 + 64
                            nc.tensor.matmul(
                                s_ps[h2][:, kti, :],
                                lhsT=KT[p0:p1, pr, kt * 128:(kt + 1) * 128],
                                rhs=QT[p0:p1, pr, q0:q1],
                                start=True, stop=not diag,
                            )
                        if diag:
                            jj = kt - qc * 4
                            msl = h["maskB_t"][:, 384 - jj * 128:896 - jj * 128]
                            for h2 in (0, 1):
                                nc.tensor.matmul(
                                    s_ps[h2][:, kti, :],
                                    lhsT=h["ident_r"][:],
                                    rhs=msl,
                                    start=False, stop=True,
                                )
                    p_sb = [
                        pP.tile([128, 2, 512], f32r, tag="p", name="p0"),
                        pP.tile([128, 2, 512], f32r, tag="p", name="p1"),
                    ]
                    for h2 in (0, 1):
                        if phase == "exp0":
                            nc.scalar.activation(
                                p_sb[h2][:, :nkt, 0:1],
                                s_ps[h2][:, :nkt, 0:1],
                                EXP, scale=0.125,
                            )
                        else:
                            nc.scalar.activation(
                                p_sb[h2][:, :nkt, :], s_ps[h2][:, :nkt, :],
                                EXP, scale=0.125,
                            )
                    if phase != "noav":
                        for kti in range(nkt):
                            kt = kb + kti
                            for h2 in (0, 1):
                                hh = pr * 2 + h2
                                nc.tensor.matmul(
                                    av[h2][:],
                                    lhsT=Vp[:, kt, hh, :],
                                    rhs=p_sb[h2][:, kti, :],
                                    start=(kt == 0), stop=(kt == ktmax - 1),
                                )
                if phase != "noav":
                    # normalize: aoT[head rows, chunk] = av[0:64] / sums.
                    # Copy PSUM->SBUF first so the AV bank frees quickly.
                    for h2 in (0, 1):
                        av_sb = small.tile([65, 512], f32, tag="avsb",
                                           name="av_sb")
                        nc.vector.tensor_copy(av_sb[:], av[h2][:])
                        r_t = small.tile([1, 512], f32, tag="r", name="r_t")
                        nc.vector.reciprocal(r_t[:], av_sb[64:65, :])
                        R_t = small.tile([64, 512], f32, tag="R", name="R_t")
                        nc.gpsimd.partition_broadcast(R_t[:], r_t[:])
                        nc.vector.tensor_mul(
                            aoT[h2 * 64:h2 * 64 + 64, pr, q0:q1],
                            av_sb[0:64, :], R_t[:],
                        )
            # out-projection for this chunk's four n-tiles
            for nt in range(qc * 4, qc * 4 + 4):
                o_sb = pout.tile([128, D], f32, tag="o", name="o_sb")
                for dc in range(2):
                    ps_o = ps_op.tile([128, 512], f32, tag="o", name="ps_o")
                    for jt in range(4):
                        nc.tensor.matmul(
                            ps_o[:],
                            lhsT=aoT[:, jt, nt * 128:(nt + 1) * 128],
                            rhs=woT_t[:, jt, dc * 512:(dc + 1) * 512],
                            start=(jt == 0), stop=(jt == 3),
                        )
                    nc.vector.tensor_copy(
                        o_sb[:, dc * 512:(dc + 1) * 512], ps_o[:]
                    )
                nc.sync.dma_start(out[nt * 128:(nt + 1) * 128, :], o_sb[:])


def _build_nc(reps=1, phase="all"):
    nc = bacc.Bacc(None, target_bir_lowering=False)
    h = {}
    for nm in ("xq", "xk", "xv"):
        h[nm] = nc.declare_dram_parameter(nm, [N, D], f32, isOutput=False)
    for nm in ("wqT", "wkT", "wvT"):
        h[nm] = nc.declare_dram_parameter(nm, [D, DH], f32, isOutput=False)
    h["woT"] = nc.declare_dram_parameter("woT", [DH, D], f32, isOutput=False)
    h["maskB"] = nc.declare_dram_parameter("maskB", [128, 896], f32,
                                           isOutput=False)
    h["ident"] = nc.declare_dram_parameter("ident", [128, 128], f32,
                                           isOutput=False)
    h["out"] = nc.declare_dram_parameter("out", [N, D], f32, isOutput=True)

    with tile.TileContext(nc) as tc:
        with (
            tc.tile_pool(name="consts", bufs=1) as consts,
            tc.tile_pool(name="qt", bufs=1) as qt_pool,
            tc.tile_pool(name="kt", bufs=1) as kt_pool,
            tc.tile_pool(name="vp", bufs=1) as vp_pool,
        ):
            h["ident_t"] = consts.tile([128, 128], f32, name="ident_t")
            nc.sync.dma_start(h["ident_t"][:], h["ident"][:])
            h["ident_r"] = consts.tile([128, 128], f32r, name="ident_r")
            nc.sync.dma_start(h["ident_r"][:], h["ident"][:].bitcast(f32r))
            h["maskB_t"] = consts.tile([128, 896], f32r, name="maskB_t")
            nc.sync.dma_start(h["maskB_t"][:], h["maskB"][:].bitcast(f32r))
            h["woT_t"] = consts.tile([128, 4, D], f32r, name="woT_t")
            nc.sync.dma_start(
                h["woT_t"][:],
                h["woT"][:].bitcast(f32r).rearrange("(o p) f -> p o f", p=128),
            )

            for rep in range(reps):
                h["QT"] = qt_pool.tile([128, 4, N], f32r, tag="QT", name="QT")
                h["KT"] = kt_pool.tile([128, 4, N], f32r, tag="KT", name="KT")
                h["Vp"] = vp_pool.tile([128, NT, 8, 65], f32r, tag="Vp",
                                       name="Vp")
                if phase == "attn":
                    nc.sync.dma_start(
                        h["QT"][:].rearrange("p a b -> p (a b)")
                        .rearrange("p (o f) -> p o f", o=8),
                        _junk3(h["xq"]),
                    )
                    nc.sync.dma_start(
                        h["KT"][:].rearrange("p a b -> p (a b)")
                        .rearrange("p (o f) -> p o f", o=8),
                        _junk3(h["xk"]),
                    )
                    nc.sync.dma_start(
                        h["Vp"][:].rearrange("p a b c -> p a (b c)"),
                        h["xv"][:].bitcast(f32r)
                        .rearrange("(o p) f -> p o f", p=128)[:, :, 0:520],
                    )
                else:
                    _phase1(nc, tc, rep, h, phase)

                if phase == "p1":
                    with tc.tile_pool(name=f"p1o{rep}", bufs=1) as p1o:
                        o_p1 = p1o.tile([128, D], f32, name="o_p1")
                        nc.vector.tensor_copy(
                            o_p1[:], h["QT"][:, 0, 0:1024].bitcast(f32))
                        nc.vector.tensor_copy(
                            o_p1[:, 0:1], h["Vp"][:, 0, 0, 0:1].bitcast(f32))
                        nc.vector.tensor_copy(
                            o_p1[:, 1:2], h["KT"][:, 0, 0:1].bitcast(f32))
                        nc.sync.dma_start(h["out"][0:128, :], o_p1[:])
                else:
                    _phase2(nc, tc, rep, h, phase)
    nc.compile()
    return nc


_NC = None


def _get_nc():
    global _NC
    if _NC is None:
        _NC = _build_nc()
    return _NC


def _make_in_maps(q, k, v, Wq, Wk, Wv, Wo):
    q = np.asarray(q, np.float32)
    k = np.asarray(k, np.float32)
    v = np.asarray(v, np.float32)
    Wq = np.asarray(Wq, np.float32)
    Wk = np.asarray(Wk, np.float32)
    Wv = np.asarray(Wv, np.float32)
    Wo = np.asarray(Wo, np.float32)

    kk = np.arange(128)[:, None]
    mm = np.arange(896)[None, :]
    maskB = np.where(kk <= mm - 384, 0.0, NEG).astype(np.float32)
    ident = np.eye(128, dtype=np.float32)

    in_maps = []
    for c in range(8):
        b, hh = divmod(c, 2)
        sl = slice(hh * DH, (hh + 1) * DH)
        in_maps.append({
            "xq": np.ascontiguousarray(q[b]),
            "xk": np.ascontiguousarray(k[b]),
            "xv": np.ascontiguousarray(v[b]),
            "wqT": np.ascontiguousarray(Wq[sl, :].T),
            "wkT": np.ascontiguousarray(Wk[sl, :].T),
            "wvT": np.ascontiguousarray(Wv[sl, :].T),
            "woT": np.ascontiguousarray(Wo[:, sl].T),
            "maskB": maskB,
            "ident": ident,
        })
    return in_maps


def kernel(q, k, v, Wq, Wk, Wv, Wo):
    nc = _get_nc()
    in_maps = _make_in_maps(q, k, v, Wq, Wk, Wv, Wo)
    res = run_bass_kernel_spmd(nc, in_maps, core_ids=list(range(8)))
    out = np.empty((B, N, D), np.float32)
    for b in range(B):
        out[b] = res.results[2 * b]["out"] + res.results[2 * b + 1]["out"]
    return out
